# revision 31
# baseline (speedup 1.0000x reference)
"""Trainium2 Bass kernel for nn_DecoderBlockMoE (MoE decoder block, 8 NeuronCores).

Strategy (4 SPMD launches + host glue as the all-to-alls):
  L1 (row-slab parallel, bf16): deferred-rmsnorm latent projections + RoPE.
     Host pre-transposes x; 1/rms folded into the z-psum evacuation; cont/rot
     emitted as separate tensors (host interleaves heads for L2).
  L2 (head-parallel, 2 heads/core): minimal-causal bf16 scores [kv,q], exp split
     between ScalarE (exact) and VectorE (Schraudolph exp2 on far-past blocks),
     V-stationary AV accumulation with a ones-column for sumexp; un-normalized
     [65, q] outputs; softmax divide on host.
  L3 (row-slab): bf16 Wout astat + residual + rmsnorm2 + TRUE-fp32 gate logits
     + shared expert in fp8e4m3 DoubleRow (weights x512, acts x8, swig x16).
  host: exact top-k routing / capacity selection (numpy, fp32 logits).
  L4 (expert-parallel): 7 routed experts, fp8e4m3 DoubleRow SwiGLU.
All stages start with a ~40-matmul warmup burst to latch the PE HAM clock gate
at 2.4 GHz; weights are host-prearranged for single contiguous DMAs.
Accuracy: ~1.2e-2 rel (fp8 expert noise; routing decisions are bit-exact fp32).
"""
import numpy as np
import ml_dtypes
import concourse.bass as bass
import concourse.mybir as mybir
import concourse.tile as tile
from concourse import bacc
from concourse.bass_utils import run_bass_kernel_spmd
from concourse.masks import make_identity



# ================= common.py =================


B, S, D = 2, 2048, 1024
H, HD = 16, 64
ROT, CONT = 32, 32
LQ, LKV = 512, 256
FF = 1024
NR, TOPK = 7, 2
CAPACITY = 585
EPS = 1e-6
T = B * S
NCORES = 8
SLAB = T // NCORES          # 512 rows per core in L1/L3
HPC = H // NCORES           # 2 heads per core in L2
NCH = S // 128              # 16 kv chunks per batch

def rotary_tables():
    inv_freq = 1.0 / (10000.0 ** (np.arange(0, ROT, 2, dtype=np.float32) / ROT))
    t = np.arange(S, dtype=np.float32)
    freqs = t[:, None] * inv_freq[None, :]
    emb = np.concatenate([freqs, freqs], axis=-1)  # [S, ROT]
    return np.cos(emb).astype(np.float32), np.sin(emb).astype(np.float32)

def fold_rot_weights(Wrot):
    """Wrot [L, H*2*ROT] -> (W1 [L, H*ROT], W2 [L, H*ROT]) where
    q_rot = (z@W1)*cos + (z@W2)*sin, with W1 = first ROT cols per head,
    W2 = rotate_half folded: W2[:, d] = -W1h[:, d+16] d<16 else W1h[:, d-16]."""
    L = Wrot.shape[0]
    Wr = Wrot.reshape(L, H, 2 * ROT)[:, :, :ROT]      # [L, H, 32]
    W2 = np.concatenate([-Wr[:, :, ROT // 2:], Wr[:, :, :ROT // 2]], axis=2)
    return (np.ascontiguousarray(Wr.reshape(L, H * ROT)),
            np.ascontiguousarray(W2.reshape(L, H * ROT)))

def interleave_heads_cont(W):
    """W [L, H*HD] -> keep first CONT cols per head -> [L, H*CONT]"""
    L = W.shape[0]
    return np.ascontiguousarray(W.reshape(L, H, HD)[:, :, :CONT].reshape(L, H * CONT))


# ================= npref.py =================

"""Pure-numpy mirror of reference.py (fp32), used by test.py and as generic fallback."""

def np_reference(x, causal_mask, Wq_lat, Wkv_lat, Wrot_q, Wrot_k, Wq_up, Wk_up, Wv_up,
                 Wout, norm1_w, norm2_w, Ws1, Ws2, Wr1, Wr2, Wgate, expert_bias):
    B, S, D = x.shape
    H, HD = 16, 64
    ROT, CONT = 32, 32
    FF = 1024
    NR, TOPK = 7, 2
    CAP = max(1, int(1.0 * B * S / NR))
    EPS = 1e-6
    f32 = np.float32

    def rms(t, w):
        return (t / np.sqrt((t * t).mean(-1, keepdims=True) + EPS) * w).astype(f32)

    def rotate_half(t):
        t1, t2 = t[..., :ROT // 2], t[..., ROT // 2:]
        return np.concatenate([-t2, t1], -1)

    x = x.astype(f32)
    xn = rms(x, norm1_w)
    zq = xn @ Wq_lat
    zkv = xn @ Wkv_lat
    qr = (zq @ Wrot_q).reshape(B, S, H, 2 * ROT)[..., :ROT].transpose(0, 2, 1, 3)
    kr = (zkv @ Wrot_k).reshape(B, S, H, 2 * ROT)[..., :ROT].transpose(0, 2, 1, 3)
    qc = (zq @ Wq_up).reshape(B, S, H, HD).transpose(0, 2, 1, 3)
    kc = (zkv @ Wk_up).reshape(B, S, H, HD).transpose(0, 2, 1, 3)
    v = (zkv @ Wv_up).reshape(B, S, H, HD).transpose(0, 2, 1, 3)
    inv = 1.0 / (10000.0 ** (np.arange(0, ROT, 2, dtype=f32) / ROT))
    t = np.arange(S, dtype=f32)
    fr = t[:, None] * inv[None, :]
    emb = np.concatenate([fr, fr], -1)
    cos, sin = np.cos(emb)[None, None].astype(f32), np.sin(emb)[None, None].astype(f32)
    qrot = qr * cos + rotate_half(qr) * sin
    krot = kr * cos + rotate_half(kr) * sin
    q = np.concatenate([qc[..., :CONT], qrot], -1)
    k = np.concatenate([kc[..., :CONT], krot], -1)
    out = np.zeros((B, H, S, HD), f32)
    for b in range(B):
        for h in range(H):
            sc = (q[b, h] @ k[b, h].T) / np.sqrt(HD).astype(f32) + causal_mask[0, 0]
            sc = sc - sc.max(-1, keepdims=True)
            e = np.exp(sc)
            out[b, h] = (e @ v[b, h]) / e.sum(-1, keepdims=True)
    o = out.transpose(0, 2, 1, 3).reshape(B, S, D) @ Wout
    x1 = x + o
    xn2 = rms(x1, norm2_w)
    flat = xn2.reshape(B * S, D)
    T = B * S
    h = flat @ Ws1
    h1, h2 = h[:, :FF], h[:, FF:]
    shared = (h1 * (h2 / (1 + np.exp(-h2)))) @ Ws2
    aff = 1.0 / (1.0 + np.exp(-(flat @ Wgate + expert_bias)))
    ord2 = np.argsort(-aff, axis=1, kind="stable")[:, :TOPK]
    member = np.zeros((T, NR), bool)
    member[np.arange(T)[:, None], ord2] = True
    pri = np.where(member, aff, -np.inf).astype(f32)
    order = np.argsort(-pri, axis=0, kind="stable")[:CAP]
    vals = pri[order, np.arange(NR)[None, :]]
    weights = np.where(np.isfinite(vals), vals, 0.0).astype(f32)
    routed = np.zeros((T, D), f32)
    for e_ in range(NR):
        g = flat[order[:, e_]]
        hh = g @ Wr1[e_]
        hh1, hh2 = hh[:, :FF], hh[:, FF:]
        eo = (hh1 * (hh2 / (1 + np.exp(-hh2)))) @ Wr2[e_]
        np.add.at(routed, order[:, e_], eo * weights[:, e_][:, None])
    return (x1 + (shared + routed).reshape(B, S, D)).astype(f32)


# ================= hostprep.py =================


def prep_shared(inputs):
    """Host-side weight prep shared by all cores. Returns dict of prepped arrays."""
    w1 = inputs["norm1_w"].astype(np.float32)
    Wq_lat = (w1[:, None] * inputs["Wq_lat"]).astype(np.float32)
    Wkv_lat = (w1[:, None] * inputs["Wkv_lat"]).astype(np.float32)
    Wrq1, Wrq2 = fold_rot_weights(inputs["Wrot_q"].astype(np.float32))
    Wrk1, Wrk2 = fold_rot_weights(inputs["Wrot_k"].astype(np.float32))
    Wq_cont = interleave_heads_cont(inputs["Wq_up"].astype(np.float32))
    Wk_cont = interleave_heads_cont(inputs["Wk_up"].astype(np.float32))
    cos, sin = rotary_tables()   # [S, 32]
    return dict(Wq_lat=Wq_lat, Wkv_lat=Wkv_lat, Wrq1=Wrq1, Wrq2=Wrq2,
                Wrk1=Wrk1, Wrk2=Wrk2, Wq_cont=Wq_cont, Wk_cont=Wk_cont,
                Wv_up=inputs["Wv_up"].astype(np.float32), cos=cos, sin=sin)

def _prearrange(W):
    """[K, M] -> contiguous [128, (K/128)*M] with chunk-major free dim."""
    K, M = W.shape
    nk = K // 128
    return np.ascontiguousarray(
        W.reshape(nk, 128, M).transpose(1, 0, 2).reshape(128, nk * M))

def l1_in_maps(inputs, shared):
    bf16 = ml_dtypes.bfloat16
    x = np.ascontiguousarray(inputs["x"].astype(np.float32).reshape(T, D))
    cos, sin = shared["cos"], shared["sin"]
    wb = {k: _prearrange(shared[k]).astype(bf16) for k in
          ("Wq_lat", "Wkv_lat", "Wq_cont", "Wk_cont", "Wv_up",
           "Wrq1", "Wrq2", "Wrk1", "Wrk2")}
    maps = []
    for c in range(NCORES):
        r0 = c * SLAB
        pos0 = r0 % S
        cos_fm = np.tile(cos[pos0:pos0 + SLAB, :].T, (4, 1))  # [128, 512]
        sin_fm = np.tile(sin[pos0:pos0 + SLAB, :].T, (4, 1))
        m = dict(
            xT_slab=_prearrange(np.ascontiguousarray(x[r0:r0 + SLAB].T)).astype(bf16),
            cos4=np.ascontiguousarray(cos_fm).astype(bf16),
            sin4=np.ascontiguousarray(sin_fm).astype(bf16),
            **wb,
        )
        maps.append(m)
    return maps

def l1_mirror(inputs, shared, c):
    """Numpy mirror of L1 outputs for core c (fp32)."""
    x = inputs["x"].astype(np.float32).reshape(T, D)[c * SLAB:(c + 1) * SLAB]
    rms = np.sqrt((x * x).mean(-1, keepdims=True) + EPS)
    xn = x / rms
    z_q = xn @ shared["Wq_lat"]
    z_kv = xn @ shared["Wkv_lat"]
    pos0 = (c * SLAB) % S
    cos = shared["cos"][pos0:pos0 + SLAB]  # [512, 32]
    sin = shared["sin"][pos0:pos0 + SLAB]

    def qk(z, Wcont, Wr1, Wr2):
        contall = z @ Wcont            # [512, 16*32]
        r1 = z @ Wr1
        r2 = z @ Wr2
        out = np.zeros((8, 128, SLAB), np.float32)
        for h in range(H):
            cont = contall[:, h * 32:(h + 1) * 32]
            rot = r1[:, h * 32:(h + 1) * 32] * cos + r2[:, h * 32:(h + 1) * 32] * sin
            tl, base = h // 2, (h % 2) * 64
            out[tl, base:base + 32] = cont.T
            out[tl, base + 32:base + 64] = rot.T
        return out

    qT = qk(z_q, shared["Wq_cont"], shared["Wrq1"], shared["Wrq2"])
    kT = qk(z_kv, shared["Wk_cont"], shared["Wrk1"], shared["Wrk2"])
    v = z_kv @ shared["Wv_up"]         # [512, 1024]
    v_out = np.zeros((4, 128, 1040), np.float32)
    for r in range(4):
        blk = v[r * 128:(r + 1) * 128].reshape(128, 16, 64)
        vv = v_out[r].reshape(128, 16, 65)
        vv[:, :, :64] = blk
        vv[:, :, 64] = 1.0
    return qT, kT, v_out


# ================= l1.py =================

"""L1 v3: per-core token slab (512 rows), all-bf16.

Host supplies xT (feature-major bf16). rmsnorm via square + ones-matmul.
Outputs cont/rot q/k as separate [4,128,512] tensors (host interleaves into
L2 layout) + v row-major with ones column (L2 layout, unchanged).
"""

F32 = mybir.dt.float32
F32R = mybir.dt.float32r
BF16 = mybir.dt.bfloat16
AX = mybir.AxisListType.X
AF = mybir.ActivationFunctionType


def build_l1(nc):
    D, LQ, LKV = 1024, 512, 256
    R = 512
    xT_in = nc.dram_tensor("xT_slab", [128, 8 * R], BF16, kind="ExternalInput").ap()
    Wq_lat = nc.dram_tensor("Wq_lat", [128, 8 * LQ], BF16, kind="ExternalInput").ap()
    Wkv_lat = nc.dram_tensor("Wkv_lat", [128, 8 * LKV], BF16, kind="ExternalInput").ap()
    Wq_cont = nc.dram_tensor("Wq_cont", [128, 4 * 512], BF16, kind="ExternalInput").ap()
    Wk_cont = nc.dram_tensor("Wk_cont", [128, 2 * 512], BF16, kind="ExternalInput").ap()
    Wv_up = nc.dram_tensor("Wv_up", [128, 2 * D], BF16, kind="ExternalInput").ap()
    Wrq1 = nc.dram_tensor("Wrq1", [128, 4 * 512], BF16, kind="ExternalInput").ap()
    Wrq2 = nc.dram_tensor("Wrq2", [128, 4 * 512], BF16, kind="ExternalInput").ap()
    Wrk1 = nc.dram_tensor("Wrk1", [128, 2 * 512], BF16, kind="ExternalInput").ap()
    Wrk2 = nc.dram_tensor("Wrk2", [128, 2 * 512], BF16, kind="ExternalInput").ap()
    cos4 = nc.dram_tensor("cos4", [128, R], BF16, kind="ExternalInput").ap()
    sin4 = nc.dram_tensor("sin4", [128, R], BF16, kind="ExternalInput").ap()
    cq_out = nc.dram_tensor("cq", [4, 128, R], BF16, kind="ExternalOutput").ap()
    rq_out = nc.dram_tensor("rq", [4, 128, R], BF16, kind="ExternalOutput").ap()
    ck_out = nc.dram_tensor("ck", [4, 128, R], BF16, kind="ExternalOutput").ap()
    rk_out = nc.dram_tensor("rk", [4, 128, R], BF16, kind="ExternalOutput").ap()
    v_out = nc.dram_tensor("v_out", [4, 128, 1040], BF16, kind="ExternalOutput").ap()

    with tile.TileContext(nc) as tc:
        with tc.tile_pool(name="const", bufs=1) as constp, \
             tc.tile_pool(name="wpool", bufs=1) as wpool, \
             tc.tile_pool(name="xpool", bufs=1) as xpool, \
             tc.tile_pool(name="zpool", bufs=1) as zpool, \
             tc.tile_pool(name="work", bufs=3) as work, \
             tc.tile_pool(name="ps", bufs=1, space="PSUM") as psp:

            eps = constp.tile([128, 1], F32, tag="eps")
            nc.vector.memset(eps[:], 1e-6)
            ones = constp.tile([128, 1], BF16, tag="ones")
            nc.vector.memset(ones[:], 1.0)
            cos_t = constp.tile([128, R], BF16, tag="cos")
            sin_t = constp.tile([128, R], BF16, tag="sin")
            nc.sync.dma_start(out=cos_t[:], in_=cos4[:])
            nc.sync.dma_start(out=sin_t[:], in_=sin4[:])

            def load_w(W_dram, Kdim, Mdim, tag):
                """One contiguous DMA of host-prearranged [128, nk*Mdim]; return chunk APs."""
                nk = Kdim // 128
                t = wpool.tile([128, nk * Mdim], BF16, tag=f"w_{tag}", name=f"w_{tag}")
                nc.sync.dma_start(out=t[:], in_=W_dram)
                return [t[:, kc * Mdim:(kc + 1) * Mdim] for kc in range(nk)]

            # ---- load xT first (everything depends on it), rmsnorm ----
            xtt = xpool.tile([128, 8 * R], BF16, tag="xt", name="xt")
            nc.sync.dma_start(out=xtt[:], in_=xT_in)
            xts = [xtt[:, kc * R:(kc + 1) * R] for kc in range(8)]
            wql = load_w(Wq_lat, D, LQ, "ql")
            wkvl = load_w(Wkv_lat, D, LKV, "kvl")
            wqc = load_w(Wq_cont, LQ, 512, "qc")
            wqr1 = load_w(Wrq1, LQ, 512, "qr1")
            wqr2 = load_w(Wrq2, LQ, 512, "qr2")
            wkc = load_w(Wk_cont, LKV, 512, "kc")
            wkr1 = load_w(Wrk1, LKV, 512, "kr1")
            wkr2 = load_w(Wrk2, LKV, 512, "kr2")
            wv = load_w(Wv_up, LKV, D, "v")
            ssq = psp.tile([1, R], F32, tag="ssq", name="ssq")
            for kc in range(8):
                sq = work.tile([128, R], BF16, tag="sq")
                nc.scalar.square(sq[:], xts[kc])
                nc.tensor.matmul(ssq[:], ones[:], sq[:], start=(kc == 0), stop=(kc == 7))
            sr = work.tile([1, R], F32, tag="sr")
            nc.scalar.activation(sr[:], ssq[:], AF.Sqrt, bias=eps[0:1], scale=1.0 / D)
            rs = work.tile([1, R], F32, tag="rs")
            nc.vector.reciprocal(rs[:], sr[:])
            rb = constp.tile([128, R], F32, tag="rb")
            nc.gpsimd.partition_broadcast(rb[:], rs[:])

            def proj(rhs_tiles, wt, mc, name):
                """psum [128, R] = sum_kc W[kc][:, mc*128:...].T @ rhs[kc]"""
                nK = len(wt)
                ps = psp.tile([128, R], F32, tag="pp", bufs=4, name=name)
                for kc in range(nK):
                    nc.tensor.matmul(ps[:], wt[kc][:, mc * 128:(mc + 1) * 128],
                                     rhs_tiles[kc][:], start=(kc == 0), stop=(kc == nK - 1))
                return ps
            # project un-normalized xT; fold 1/rms per-token (column) at psum evac
            z_qT, z_kvT = [], []
            for mc in range(LQ // 128):
                ps = proj(xts, wql, mc, f"pzq{mc}")
                st = zpool.tile([128, R], BF16, tag=f"zq{mc}", name=f"zq{mc}")
                nc.vector.tensor_mul(st[:], ps[:], rb[:])
                z_qT.append(st)
            for mc in range(LKV // 128):
                ps = proj(xts, wkvl, mc, f"pzkv{mc}")
                st = zpool.tile([128, R], BF16, tag=f"zkv{mc}", name=f"zkv{mc}")
                nc.vector.tensor_mul(st[:], ps[:], rb[:])
                z_kvT.append(st)

            # ---- q/k cont + rot (RoPE); outputs per 4-head group tile ----
            def emit_cont_rot(zT, wc, w1, w2, c_out, r_out, tag):
                for g in range(4):
                    cont_ps = proj(zT, wc, g, f"pc_{tag}{g}")
                    ct = work.tile([128, R], BF16, tag="ct", bufs=2)
                    nc.scalar.copy(ct[:], cont_ps[:])
                    nc.sync.dma_start(out=c_out[g], in_=ct[:])
                    r1_ps = proj(zT, w1, g, f"pr1_{tag}{g}")
                    r1b = work.tile([128, R], BF16, tag="r1b", bufs=2)
                    nc.scalar.copy(r1b[:], r1_ps[:])
                    r2_ps = proj(zT, w2, g, f"pr2_{tag}{g}")
                    r2b = work.tile([128, R], BF16, tag="r2b", bufs=2)
                    nc.scalar.copy(r2b[:], r2_ps[:])
                    t1 = work.tile([128, R], BF16, tag="t1", bufs=2)
                    nc.vector.tensor_mul(t1[:], r1b[:], cos_t[:])
                    t2 = work.tile([128, R], BF16, tag="t2", bufs=2)
                    nc.vector.tensor_mul(t2[:], r2b[:], sin_t[:])
                    rt = work.tile([128, R], BF16, tag="rt", bufs=2)
                    nc.gpsimd.tensor_add(rt[:], t1[:], t2[:])
                    nc.sync.dma_start(out=r_out[g], in_=rt[:])

            emit_cont_rot(z_qT, wqc, wqr1, wqr2, cq_out, rq_out, "q")
            emit_cont_rot(z_kvT, wkc, wkr1, wkr2, ck_out, rk_out, "k")

            # ---- v row-major with ones columns ----
            for tch in range(4):
                ps = psp.tile([128, 1024], F32, tag="pv", bufs=1, name=f"pv{tch}")
                for kc in range(2):
                    for half in range(2):
                        nc.tensor.matmul(ps[:, half * 512:(half + 1) * 512],
                                         z_kvT[kc][:, tch * 128:(tch + 1) * 128],
                                         wv[kc][:, half * 512:(half + 1) * 512],
                                         start=(kc == 0), stop=(kc == 1))
                vt = work.tile([128, 1040], BF16, tag="vt", bufs=2)
                nc.vector.memset(vt[:].rearrange("p (h c) -> p h c", c=65)[:, :, 64:65], 1.0)
                nc.vector.tensor_copy(
                    vt[:].rearrange("p (h c) -> p h c", c=65)[:, :, 0:64],
                    ps[:].rearrange("p (h c) -> p h c", c=64))
                nc.sync.dma_start(out=v_out[tch], in_=vt[:])
    return nc


# ================= l2.py =================

"""L2 v3: head-parallel causal attention, minimal-causal bf16 scores + V-stationary AV.

Per slab (b, t): for each 1024-wide q group, loop kv chunks i:
  scores[128kv, w] = k_i^T q  (bf16, trimmed to q >= 128i)
  A = exp(scores/8)           (scalar exact; vector fast-exp2 on far-past blocks)
  av[65, 1024] += V_i^T A     (V stationary [128,65] with ones col -> row 64 = sumexp)
Output oh_out [2, 2, 65, 2048] f32 UN-normalized; host divides by row 64.

Inputs:
  q_in [2, 128, 2048] bf16, k_in [2, 128, 2048] bf16
  v_in [2, 2, 16, 128, 65] bf16
  tri  [128, 128] bf16
"""

F32 = mybir.dt.float32
F32R = mybir.dt.float32r
BF16 = mybir.dt.bfloat16
I16 = mybir.dt.int16
AF = mybir.ActivationFunctionType
ALU = mybir.AluOpType

# fast exp2 constants: bits_bf16(exp(s/8)) ~= round(s * 0.125*log2(e)*128) + 16256
FEXP_MUL = 0.125 * 1.4426950408889634 * 128.0
FEXP_ADD = 16256.0


def build_l2(nc):
    S = 2048
    q_in = nc.dram_tensor("q_in", [2, 128, S], BF16, kind="ExternalInput").ap()
    k_in = nc.dram_tensor("k_in", [2, 128, S], BF16, kind="ExternalInput").ap()
    v_in = nc.dram_tensor("v_in", [2, 2, 128, 1040], BF16, kind="ExternalInput").ap()
    tri_in = nc.dram_tensor("tri", [128, 128], BF16, kind="ExternalInput").ap()
    oh_out = nc.dram_tensor("oh_out", [2, 2, 65, S], F32, kind="ExternalOutput").ap()

    with tile.TileContext(nc) as tc:
        with tc.tile_pool(name="const", bufs=1) as constp, \
             tc.tile_pool(name="qk", bufs=1) as qkp, \
             tc.tile_pool(name="vp", bufs=1) as vp, \
             tc.tile_pool(name="at", bufs=6) as atp, \
             tc.tile_pool(name="ot", bufs=3) as otp, \
             tc.tile_pool(name="scp", bufs=3, space="PSUM") as scpool, \
             tc.tile_pool(name="avp", bufs=1, space="PSUM") as avpool:

            tri = constp.tile([128, 128], BF16, tag="tri")
            nc.sync.dma_start(out=tri[:], in_=tri_in[:])
            # HAM warmup: ~40 dependency-free matmuls back-to-back ramp PE to 2.4GHz
            wm = scpool.tile([128, 1024], F32, tag="sc", name="warm")
            for _ in range(40):
                nc.tensor.matmul(wm[:, 0:128], tri[:], tri[:], start=True, stop=True)
            q_sb, k_sb, v_sb = {}, {}, {}
            for b in range(2):
                q_sb[b] = qkp.tile([128, S], BF16, tag=f"q{b}", name=f"q{b}")
                nc.sync.dma_start(out=q_sb[b][:], in_=q_in[b])
                k_sb[b] = qkp.tile([128, S], BF16, tag=f"k{b}", name=f"k{b}")
                nc.sync.dma_start(out=k_sb[b][:], in_=k_in[b])
                for t in range(2):
                    v_sb[(b, t)] = vp.tile([128, 16 * 65], BF16, tag=f"v{b}{t}", name=f"v{b}{t}")
                    nc.sync.dma_start(out=v_sb[(b, t)][:], in_=v_in[b, t])

            for b in range(2):
                for t in range(2):
                    kh = k_sb[b][t * 64:(t + 1) * 64, :]
                    qh = q_sb[b][t * 64:(t + 1) * 64, :]
                    vh = v_sb[(b, t)]
                    for qg in range(2):
                        av = avpool.tile([65, 1024], F32, tag="av", name=f"av{b}{t}{qg}")
                        imax = min(16, 8 * qg + 8)
                        scs, ats = {}, {}

                        def emit_sc(i, qg=qg, scs=scs):
                            off = max(0, 128 * i - 1024 * qg)
                            sc = scpool.tile([128, 1024], F32, tag="sc",
                                             name=f"sc{b}{t}{qg}{i}")
                            # scores, split at psum bank boundary (512 f32 cols)
                            for p0, p1 in ((off, 512), (max(off, 512), 1024)):
                                if p0 >= p1:
                                    continue
                                nc.tensor.matmul(
                                    sc[:, p0:p1], kh[:, i * 128:(i + 1) * 128],
                                    qh[:, 1024 * qg + p0:1024 * qg + p1],
                                    start=True, stop=True)
                            scs[i] = (sc, off)

                        def emit_exp(i, qg=qg, scs=scs, ats=ats):
                            sc, off = scs.pop(i)
                            at = atp.tile([128, 1024], BF16, tag="at",
                                          name=f"at{b}{t}{qg}{i}")
                            if qg == 1 and i < 6:
                                # far-past block: fast exp2 on vector engine
                                nc.vector.tensor_scalar(
                                    at[:, off:1024].bitcast(I16), sc[:, off:1024],
                                    FEXP_MUL, FEXP_ADD, ALU.mult, ALU.add)
                            else:
                                nc.scalar.activation(at[:, off:1024], sc[:, off:1024],
                                                     AF.Exp, scale=0.125)
                            if 128 * i >= 1024 * qg:
                                # diagonal chunk: mask strict upper triangle
                                dc = 128 * i - 1024 * qg
                                nc.vector.tensor_mul(at[:, dc:dc + 128],
                                                     at[:, dc:dc + 128], tri[:])
                            ats[i] = (at, off)

                        def emit_av(i, qg=qg, imax=imax, av=av, ats=ats):
                            at, off = ats.pop(i)
                            for p0, p1 in ((off, 512), (max(off, 512), 1024)):
                                if p0 >= p1:
                                    continue
                                last_a = (i == min(imax, 8 * qg + 4) - 1)
                                nc.tensor.matmul(
                                    av[:, p0:p1], vh[:, i * 65:(i + 1) * 65],
                                    at[:, p0:p1],
                                    start=(i == 0),
                                    stop=(last_a if p1 <= 512 else i == imax - 1))

                        # software-pipelined emission: engine queues are in-order,
                        # so keep independent score work between exp -> AV edges
                        for i in range(imax + 2):
                            if i < imax:
                                emit_sc(i)
                            if 0 <= i - 1 < imax:
                                emit_exp(i - 1)
                            if i - 2 >= 0:
                                emit_av(i - 2)
                        oh = otp.tile([65, 1024], F32, tag="oh", name=f"oh{b}{t}{qg}")
                        nc.vector.tensor_copy(oh[:], av[:])
                        nc.sync.dma_start(
                            out=oh_out[b, t, :, 1024 * qg:1024 * (qg + 1)], in_=oh[:])
    return nc


# ================= l3.py =================

"""L3 v3: row-slab: Wout (bf16 astat, wide rhs) + residual + rmsnorm2 + gate (fp32)
+ shared expert in fp8e4m3 DoubleRow (weight-stationary h -> feature-major swig,
astat eout; no swig transposes).

fp8 scaling: xn2 x8, Ws1 x512 (host), h_psum = 4096*h_true.
swig_fp8 = 16*swig_true; Ws2 x512 (host); eout_psum = 8192*eout_true.
"""

F32 = mybir.dt.float32
F32R = mybir.dt.float32r
BF16 = mybir.dt.bfloat16
FP8 = mybir.dt.float8e4
AX = mybir.AxisListType.X
AF = mybir.ActivationFunctionType
ALU = mybir.AluOpType
DR = mybir.MatmulPerfMode.DoubleRow
D = 1024
XN2_S = 8.0           # fp8 activation scale
W_S = 512.0           # fp8 weight scale
SW_S = 16.0           # fp8 swig scale


def build_l3(nc):
    R = 512
    x_in = nc.dram_tensor("x_slab", [R, D], F32, kind="ExternalInput").ap()
    ocT_in = nc.dram_tensor("ocT", [128, 8 * R], BF16, kind="ExternalInput").ap()
    Wout_in = nc.dram_tensor("Wout", [128, 8 * D], BF16, kind="ExternalInput").ap()
    Wgate_in = nc.dram_tensor("Wgate", [128, 8 * 7], F32, kind="ExternalInput").ap()
    Ws1_in = nc.dram_tensor("Ws1d", [128, 8, 2048], FP8, kind="ExternalInput").ap()
    Ws2_in = nc.dram_tensor("Ws2d", [128, 8, D], FP8, kind="ExternalInput").ap()
    x1_out = nc.dram_tensor("x1_out", [R, D], F32, kind="ExternalOutput").ap()
    xn2_out = nc.dram_tensor("xn2_out", [R, D], BF16, kind="ExternalOutput").ap()
    shared_out = nc.dram_tensor("shared_out", [R, D], F32, kind="ExternalOutput").ap()
    logits_out = nc.dram_tensor("logits_out", [7, R], F32, kind="ExternalOutput").ap()

    with tile.TileContext(nc) as tc:
        with tc.tile_pool(name="const", bufs=1) as constp, \
             tc.tile_pool(name="wpool", bufs=1) as wpool, \
             tc.tile_pool(name="apool", bufs=1) as apool, \
             tc.tile_pool(name="work", bufs=3) as work, \
             tc.tile_pool(name="ps", bufs=4, space="PSUM") as psp:

            ident_f = constp.tile([128, 128], F32, tag="ident_f")
            make_identity(nc, ident_f)
            eps = constp.tile([128, 1], F32, tag="eps")
            nc.vector.memset(eps[:], 1e-6)

            wmt = constp.tile([128, 128], BF16, tag="wmt")
            nc.vector.memset(wmt[:], 0.5)
            wmp = psp.tile([128, 512], F32, tag="ph", bufs=4, name="warm")
            for _ in range(40):
                nc.tensor.matmul(wmp[:, 0:128], wmt[:], wmt[:], start=True, stop=True)
            ocT_t = apool.tile([128, 8 * R], BF16, tag="ocT", name="ocT")
            nc.sync.dma_start(out=ocT_t[:, 0:4 * R], in_=ocT_in[:, 0:4 * R])
            nc.sync.dma_start(out=ocT_t[:, 4 * R:8 * R], in_=ocT_in[:, 4 * R:8 * R])
            ocT = [ocT_t[:, kc * R:(kc + 1) * R] for kc in range(8)]
            wout_t = wpool.tile([128, 8 * D], BF16, tag="w_wo", name="w_wo")
            nc.sync.dma_start(out=wout_t[:], in_=Wout_in[:])
            wout = [wout_t[:, kc * D:(kc + 1) * D] for kc in range(8)]
            ws1 = wpool.tile([128, 8 * 2048], FP8, tag="ws1", name="ws1")
            nc.sync.dma_start(out=ws1[:].rearrange("p (k m) -> p k m", k=8), in_=Ws1_in[:])
            ws1 = ws1[:].rearrange("p (k m) -> p k m", k=8)
            ws2 = wpool.tile([128, 8 * D], FP8, tag="ws2", name="ws2")
            nc.sync.dma_start(out=ws2[:].rearrange("p (k m) -> p k m", k=8), in_=Ws2_in[:])
            ws2 = ws2[:].rearrange("p (k m) -> p k m", k=8)

            # ---- delta = ocT.T @ Wout (bf16 astat, 1024-wide rhs); residual; rmsnorm2 ----
            xn2T = [apool.tile([128, R], F32, tag=f"xn2T{kc}", name=f"xn2T{kc}") for kc in range(8)]
            xn2t8 = apool.tile([128, 8 * R], FP8, tag="xn2t8", name="xn2t8")
            xn2t8 = xn2t8[:].rearrange("p (k t) -> p k t", k=8)
            xns = [apool.tile([128, D], F32, tag=f"xn_{r}", name=f"xn_{r}") for r in range(4)]
            for rb in range(4):
                ps = psp.tile([128, 1024], F32, tag="pp", bufs=2, name=f"pd{rb}")
                for kc in range(8):
                    for half in range(2):
                        nc.tensor.matmul(ps[:, half * 512:(half + 1) * 512],
                                         ocT[kc][:, rb * 128:(rb + 1) * 128],
                                         wout[kc][:, half * 512:(half + 1) * 512],
                                         start=(kc == 0), stop=(kc == 7))
                xt = work.tile([128, D], F32, tag="xt", bufs=2)
                nc.sync.dma_start(out=xt[:], in_=x_in[rb * 128:(rb + 1) * 128, :])
                x1 = work.tile([128, D], F32, tag="x1w", bufs=2)
                nc.vector.tensor_add(x1[:], xt[:], ps[:])
                nc.sync.dma_start(out=x1_out[rb * 128:(rb + 1) * 128, :], in_=x1[:])
                sq = work.tile([128, D], F32, tag="sq", bufs=2)
                nc.scalar.square(sq[:], x1[:])
                ssq = work.tile([128, 1], F32, tag="ssq")
                nc.vector.reduce_sum(ssq[:], sq[:], axis=AX)
                sr = work.tile([128, 1], F32, tag="sr")
                nc.scalar.activation(sr[:], ssq[:], AF.Sqrt, bias=eps[:], scale=1.0 / D)
                rs = work.tile([128, 1], F32, tag="rs")
                nc.vector.reciprocal(rs[:], sr[:])
                nc.vector.tensor_scalar_mul(xns[rb][:], x1[:], rs[:])
            # transposes to feature-major fp32 (for gate) + fp8 (for shared h)
            for kc in range(8):
                pt = psp.tile([128, 512], F32, tag="ph", bufs=4, name=f"ptn{kc}")
                for rb in range(4):
                    nc.tensor.transpose(pt[:, rb * 128:(rb + 1) * 128],
                                        xns[rb][:, kc * 128:(kc + 1) * 128], ident_f[:])
                nc.vector.tensor_copy(xn2T[kc][:], pt[:])
                nc.scalar.activation(xn2t8[:, kc, :], pt[:], AF.Copy, scale=XN2_S)

            # ---- shared expert fp8 DoubleRow: h weight-stationary (feature-major) ----
            swig8 = apool.tile([128, 8 * R], FP8, tag="swig8", name="swig8")
            swig8 = swig8[:].rearrange("p (k t) -> p k t", k=8)
            for m in range(8):     # ff chunk of swig (h1 chunk m, h2 chunk 8+m)
                ps1 = psp.tile([128, R], F32, tag="ph", bufs=4, name=f"ph1_{m}")
                ps2 = psp.tile([128, R], F32, tag="ph", bufs=4, name=f"ph2_{m}")
                for po in range(0, 8, 2):
                    nc.tensor.matmul(ps1[:], ws1[:, po:po + 2, m * 128:(m + 1) * 128],
                                     xn2t8[:, po:po + 2, :], start=(po == 0),
                                     stop=(po == 6), perf_mode=DR)
                    nc.tensor.matmul(ps2[:], ws1[:, po:po + 2, (8 + m) * 128:(9 + m) * 128],
                                     xn2t8[:, po:po + 2, :], start=(po == 0),
                                     stop=(po == 6), perf_mode=DR)
                sg = work.tile([128, R], F32, tag="sg", bufs=2)
                nc.scalar.activation(sg[:], ps2[:], AF.Sigmoid, scale=1.0 / (XN2_S * W_S))
                sil = work.tile([128, R], BF16, tag="sil", bufs=2)
                nc.vector.scalar_tensor_tensor(
                    sil[:], ps2[:], SW_S / (XN2_S * W_S), sg[:], ALU.mult, ALU.mult)
                nc.vector.scalar_tensor_tensor(
                    swig8[:, m, :], ps1[:], 1.0 / (XN2_S * W_S), sil[:], ALU.mult, ALU.mult)
            # ---- gate logits: fp32 exact ----
            wg_t = wpool.tile([128, 8 * 7], F32, tag="w_wg", name="w_wg")
            nc.sync.dma_start(out=wg_t[:], in_=Wgate_in[:])
            wg = [wg_t[:, kc * 7:(kc + 1) * 7] for kc in range(8)]
            psg = psp.tile([7, R], F32, tag="pp", bufs=2, name="psg")
            for kc in range(8):
                nc.tensor.matmul(psg[:], wg[kc][:], xn2T[kc][:], start=(kc == 0), stop=(kc == 7))
            lg = work.tile([7, R], F32, tag="lg")
            nc.vector.tensor_copy(lg[:], psg[:])
            nc.sync.dma_start(out=logits_out[:], in_=lg[:])

            # eout astat DR: lhsT = swig8 token-chunk, rhs = Ws2 rows
            for rb in range(4):
                ps = psp.tile([128, 1024], F32, tag="pp", bufs=2, name=f"pe{rb}")
                for po in range(0, 8, 2):
                    for half in range(2):
                        nc.tensor.matmul(
                            ps[:, half * 512:(half + 1) * 512],
                            swig8[:, po:po + 2, rb * 128:(rb + 1) * 128],
                            ws2[:, po:po + 2, half * 512:(half + 1) * 512],
                            start=(po == 0), stop=(po == 6), perf_mode=DR)
                so = work.tile([128, D], F32, tag="so", bufs=2)
                nc.scalar.activation(so[:], ps[:], AF.Copy, scale=1.0 / (SW_S * W_S))
                nc.sync.dma_start(out=shared_out[rb * 128:(rb + 1) * 128, :], in_=so[:])
            for rb in range(4):
                xnb = work.tile([128, D], BF16, tag="xnb", bufs=2)
                nc.scalar.copy(xnb[:], xns[rb][:])
                nc.sync.dma_start(out=xn2_out[rb * 128:(rb + 1) * 128, :], in_=xnb[:])
    return nc


# ================= l4.py =================

"""L4 v3: one routed expert per core, fp8e4m3 DoubleRow.

Inputs (DR layout [128, ksub, free], k = ksub*128 + partition):
  gT8  [128, 8, 640] fp8 = 8 * gathered_tokens^T (cols 585.. zero)
  Wr1d [128, 8, 2048] fp8 = 512 * Wr1[e]
  Wr2d [128, 8, 1024] fp8 = 512 * Wr2[e]
Output: eout_out [640, 1024] f32 TOKEN-major.
"""

F32 = mybir.dt.float32
BF16 = mybir.dt.bfloat16
FP8 = mybir.dt.float8e4
AF = mybir.ActivationFunctionType
ALU = mybir.AluOpType
DR = mybir.MatmulPerfMode.DoubleRow
D, FF2, NCOL = 1024, 2048, 640
G_S = 8.0
W_S = 512.0
SW_S = 16.0


def build_l4(nc):
    gT_in = nc.dram_tensor("gT8", [128, 8, NCOL], FP8, kind="ExternalInput").ap()
    Wr1_in = nc.dram_tensor("Wr1d", [128, 8, FF2], FP8, kind="ExternalInput").ap()
    Wr2_in = nc.dram_tensor("Wr2d", [128, 8, D], FP8, kind="ExternalInput").ap()
    eout_out = nc.dram_tensor("eout_out", [NCOL, D], F32, kind="ExternalOutput").ap()

    with tile.TileContext(nc) as tc:
        with tc.tile_pool(name="wpool", bufs=1) as wpool, \
             tc.tile_pool(name="apool", bufs=1) as apool, \
             tc.tile_pool(name="work", bufs=3) as work, \
             tc.tile_pool(name="ps", bufs=2, space="PSUM") as psp:

            gT = apool.tile([128, 8 * NCOL], FP8, tag="gT", name="gT")
            gT = gT[:].rearrange("p (k t) -> p k t", k=8)
            nc.sync.dma_start(out=gT[:, 0:2, :], in_=gT_in[:, 0:2, :])
            nc.sync.dma_start(out=gT[:, 2:8, :], in_=gT_in[:, 2:8, :])
            w1 = wpool.tile([128, 8 * FF2], FP8, tag="w1", name="w1")
            w1 = w1[:].rearrange("p (k m) -> p k m", k=8)
            nc.sync.dma_start(out=w1[:, 0:4, :], in_=Wr1_in[:, 0:4, :])
            nc.sync.dma_start(out=w1[:, 4:8, :], in_=Wr1_in[:, 4:8, :])
            w2 = wpool.tile([128, 8 * D], FP8, tag="w2", name="w2")
            nc.sync.dma_start(out=w2[:].rearrange("p (k m) -> p k m", k=8), in_=Wr2_in[:])
            w2 = w2[:].rearrange("p (k m) -> p k m", k=8)

            wmt = wpool.tile([128, 128], BF16, tag="wmt", name="wmt")
            nc.vector.memset(wmt[:], 0.5)
            wmp = psp.tile([128, 512], F32, tag="ph", bufs=6, name="warm")
            for _ in range(40):
                nc.tensor.matmul(wmp[:, 0:128], wmt[:], wmt[:], start=True, stop=True)
            swig8 = apool.tile([128, 8 * NCOL], FP8, tag="swig8", name="swig8")
            swig8 = swig8[:].rearrange("p (k t) -> p k t", k=8)
            for m in range(8):   # swig ff chunk: h1 chunk m, h2 chunk 8+m
                for c0, c1 in ((0, 512), (512, NCOL)):
                    w = c1 - c0
                    ps1 = psp.tile([128, 512], F32, tag="ph", bufs=6, name=f"ph1_{m}{c0}")
                    ps2 = psp.tile([128, 512], F32, tag="ph", bufs=6, name=f"ph2_{m}{c0}")
                    for po in range(0, 8, 2):
                        nc.tensor.matmul(ps1[:, 0:w],
                                         w1[:, po:po + 2, m * 128:(m + 1) * 128],
                                         gT[:, po:po + 2, c0:c1], start=(po == 0),
                                         stop=(po == 6), perf_mode=DR)
                        nc.tensor.matmul(ps2[:, 0:w],
                                         w1[:, po:po + 2, (8 + m) * 128:(9 + m) * 128],
                                         gT[:, po:po + 2, c0:c1], start=(po == 0),
                                         stop=(po == 6), perf_mode=DR)
                    sg = work.tile([128, 512], F32, tag="sg", bufs=3)
                    nc.scalar.activation(sg[:, 0:w], ps2[:, 0:w], AF.Sigmoid,
                                         scale=1.0 / (G_S * W_S))
                    sil = work.tile([128, 512], BF16, tag="sil", bufs=3)
                    nc.vector.scalar_tensor_tensor(
                        sil[:, 0:w], ps2[:, 0:w], SW_S / (G_S * W_S), sg[:, 0:w],
                        ALU.mult, ALU.mult)
                    nc.vector.scalar_tensor_tensor(
                        swig8[:, m, c0:c1], ps1[:, 0:w], 1.0 / (G_S * W_S), sil[:, 0:w],
                        ALU.mult, ALU.mult)
            # eout astat DR: lhsT = swig8 token-chunk, rhs = Wr2 rows
            for tc_i in range(5):
                eo = work.tile([128, D], F32, tag="eo", bufs=2)
                for half in range(2):
                    ps = psp.tile([128, 512], F32, tag="ph", bufs=6, name=f"pe{tc_i}{half}")
                    for po in range(0, 8, 2):
                        nc.tensor.matmul(
                            ps[:],
                            swig8[:, po:po + 2, tc_i * 128:(tc_i + 1) * 128],
                            w2[:, po:po + 2, half * 512:(half + 1) * 512],
                            start=(po == 0), stop=(po == 6), perf_mode=DR)
                    nc.scalar.activation(eo[:, half * 512:(half + 1) * 512], ps[:],
                                         AF.Copy, scale=1.0 / (SW_S * W_S))
                nc.sync.dma_start(out=eout_out[tc_i * 128:(tc_i + 1) * 128, :], in_=eo[:])
    return nc


# ================= pipeline =================

"""Full 4-launch pipeline with host glue."""

_cache = {}

def _get(name, builder):
    if name not in _cache:
        nc = bacc.Bacc("TRN2", target_bir_lowering=False, debug=False, num_devices=8)
        builder(nc)
        nc.compile()
        _cache[name] = nc
    return _cache[name]

def run_stage(name, builder, in_maps, trace=False):
    nc = _get(name, builder)
    bk = run_bass_kernel_spmd(nc, in_maps, list(range(NCORES)), trace=trace)
    return bk

def sigmoid(x):
    return 1.0 / (1.0 + np.exp(-x.astype(np.float32), dtype=np.float32))

def route(logits_all, expert_bias):
    aff = sigmoid(logits_all + expert_bias[None, :].astype(np.float32))
    ord2 = np.argsort(-aff, axis=1, kind="stable")[:, :TOPK]
    member = np.zeros((T, NR), bool)
    member[np.arange(T)[:, None], ord2] = True
    priority = np.where(member, aff, -np.inf).astype(np.float32)
    order = np.argsort(-priority, axis=0, kind="stable")[:CAPACITY]   # [CAP, NR]
    vals = priority[order, np.arange(NR)[None, :]]
    weights = np.where(np.isfinite(vals), vals, 0.0).astype(np.float32)
    return order.T.copy(), weights.T.copy(), aff    # idx [NR, CAP], w [NR, CAP]

def full_pipeline(inputs, trace=False, timers=None):
    timers = timers if timers is not None else {}
    shared = prep_shared(inputs)
    # ---------- L1 ----------
    bk1 = run_stage("l1", build_l1, l1_in_maps(inputs, shared), trace)
    timers["l1"] = bk1.exec_time_ns
    r1 = bk1.results
    # assemble L2 inputs
    tri = (np.arange(128)[:, None] <= np.arange(128)[None, :]).astype(np.float32)
    import ml_dtypes
    tri = tri.astype(ml_dtypes.bfloat16)
    l2_maps = []
    for c in range(NCORES):
        q_in = np.zeros((2, 128, S), ml_dtypes.bfloat16)
        k_in = np.zeros((2, 128, S), ml_dtypes.bfloat16)
        v_in = np.zeros((2, 2, 128, 16, 65), ml_dtypes.bfloat16)
        for b in range(2):
            for t in range(2):
                h = 2 * c + t
                g, i = h // 4, h % 4
                for j in range(4):
                    src = r1[4 * b + j]
                    cs = slice(512 * j, 512 * (j + 1))
                    q_in[b][t * 64:t * 64 + 32, cs] = src["cq"][g][32 * i:32 * i + 32]
                    q_in[b][t * 64 + 32:t * 64 + 64, cs] = src["rq"][g][32 * i:32 * i + 32]
                    k_in[b][t * 64:t * 64 + 32, cs] = src["ck"][g][32 * i:32 * i + 32]
                    k_in[b][t * 64 + 32:t * 64 + 64, cs] = src["rk"][g][32 * i:32 * i + 32]
                for n in range(16):
                    v_in[b, t, :, n, :] = r1[4 * b + n // 4]["v_out"][n % 4][:, h * 65:(h + 1) * 65]
        l2_maps.append(dict(q_in=q_in, k_in=k_in,
                            v_in=v_in.reshape(2, 2, 128, 1040), tri=tri))
    # ---------- L2 ----------
    bk2 = run_stage("l2", build_l2, l2_maps, trace)
    timers["l2"] = bk2.exec_time_ns
    r2 = bk2.results
    out_cat = np.zeros((T, D), np.float32)
    for c in range(NCORES):
        oh = r2[c]["oh_out"]          # [2, 2, 65, 2048] un-normalized
        for b in range(2):
            for t in range(2):
                h = 2 * c + t
                ohn = oh[b, t, :64] / oh[b, t, 64:65]
                out_cat[b * S:(b + 1) * S, h * 64:(h + 1) * 64] = ohn.T
    # ---------- L3 ----------
    x = np.ascontiguousarray(inputs["x"].astype(np.float32).reshape(T, D))
    w2 = inputs["norm2_w"].astype(np.float32)
    Wgate_f = (w2[:, None] * inputs["Wgate"].astype(np.float32)).astype(np.float32)
    Ws1_f = (w2[:, None] * inputs["Ws1"].astype(np.float32)).astype(np.float32)
    Ws2 = inputs["Ws2"].astype(np.float32)
    Wout_b = _prearrange(inputs["Wout"].astype(np.float32)).astype(ml_dtypes.bfloat16)
    Wgate_p = _prearrange(Wgate_f)
    np8 = mybir.dt.np(mybir.dt.float8e4)

    def dr8(W, scale):
        """[K, M] -> DoubleRow fp8 layout [128, K//128, M], scaled."""
        K, M = W.shape
        return np.ascontiguousarray(
            (W * scale).reshape(K // 128, 128, M).transpose(1, 0, 2)).astype(np8)

    Ws1d = dr8(Ws1_f, 512.0)
    Ws2d = dr8(Ws2, 512.0)
    l3_maps = []
    for c in range(NCORES):
        r0 = c * SLAB
        l3_maps.append(dict(
            x_slab=np.ascontiguousarray(x[r0:r0 + SLAB]),
            ocT=_prearrange(np.ascontiguousarray(
                out_cat[r0:r0 + SLAB].T)).astype(ml_dtypes.bfloat16),
            Wout=Wout_b, Wgate=Wgate_p, Ws1d=Ws1d, Ws2d=Ws2d))
    bk3 = run_stage("l3", build_l3, l3_maps, trace)
    timers["l3"] = bk3.exec_time_ns
    r3 = bk3.results
    x1_all = np.concatenate([r3[c]["x1_out"] for c in range(NCORES)], axis=0)
    xn2_all = np.concatenate([r3[c]["xn2_out"] for c in range(NCORES)],
                             axis=0).astype(np.float32)
    shared_all = np.concatenate([r3[c]["shared_out"] for c in range(NCORES)], axis=0)
    logits_all = np.concatenate([r3[c]["logits_out"].T for c in range(NCORES)], axis=0)
    # ---------- routing ----------
    idx, wts, aff = route(logits_all, inputs["expert_bias"])
    flat = xn2_all * w2[None, :]
    l4_maps = []
    for c in range(NCORES):
        if c < NR:
            gp = np.zeros((640, D), np.float32)
            gp[:CAPACITY] = flat[idx[c]]
            gT8 = dr8(np.ascontiguousarray(gp.T), 8.0)
            l4_maps.append(dict(
                gT8=gT8,
                Wr1d=dr8(inputs["Wr1"][c].astype(np.float32), 512.0),
                Wr2d=dr8(inputs["Wr2"][c].astype(np.float32), 512.0)))
        else:
            l4_maps.append(dict(gT8=np.zeros((128, 8, 640), np8),
                                Wr1d=np.zeros((128, 8, 2 * FF), np8),
                                Wr2d=np.zeros((128, 8, D), np8)))
    bk4 = run_stage("l4", build_l4, l4_maps, trace)
    timers["l4"] = bk4.exec_time_ns
    r4 = bk4.results
    routed = np.zeros((T, D), np.float32)
    for e in range(NR):
        eout = r4[e]["eout_out"][:CAPACITY]            # [CAP, D] token-major
        np.add.at(routed, idx[e], eout * wts[e][:, None])
    final = x1_all + shared_all + routed
    return final.reshape(B, S, D), dict(x1=x1_all, xn2=xn2_all, aff=aff,
                                        out_cat=out_cat, shared=shared_all, routed=routed)



# ================= entry point =================

F32CONSTS_READY = True

def _is_causal_mask(mask):
    S_ = mask.shape[-1]
    m = mask.reshape(S_, S_)
    tri = np.triu(np.ones((S_, S_), bool), 1)
    return (np.all(m[~tri] == 0.0) and np.all(m[tri] <= -1e8))

def kernel(**inputs):
    inputs = {k: np.asarray(v) for k, v in inputs.items()}
    mask = inputs["causal_mask"].astype(np.float32)
    if not _is_causal_mask(mask):
        # generic fallback: exact numpy reference (correct for any mask)
        return np_reference(**{k: inputs[k].astype(np.float32) if inputs[k].dtype != np.int32 else inputs[k]
                               for k in inputs})
    out, _ = full_pipeline(inputs)
    return out.astype(np.float32)

